# revision 58
# baseline (speedup 1.0000x reference)
"""Trainium2 Bass kernel for the AtLocCriterion loss.

loss = mean(|pred - targ|)
     + 0.1 * mean(|mat2euler(chain(pose_vec2mat44(targ))) - pr_glpose|)
     + 0.01 * mean(|svdvals(weight2) - 1|)

Strategy (8 NeuronCores, pure data parallel on the batch axis); v2 pipeline
(see build_nc_v2) is active:
  - each core gets B/8 = 32768 items, laid out as [128 partitions x 256 items]
  - the host passes -tar; term1 uses an in-place CCE DMA (accum add:
    pred + (-tar)) over the tar tile after trig consumed it, then ACT Abs
    with accum_out.  No separate pred buffers, no negation pass.
  - pose math: ACT Sin at quarter/half angle (LUT domain), DVE builds
    (C, S, -S) with TT/TS ops only (STT runs 1x - avoided), then a 9-step
    descending Givens chain on the bundled (col0, col1, translation)
    vectors in bf16, entirely on the DVE.  gpsimd compute is avoided:
    the shared DVE/Pool SBUF port makes any Pool offload a large net loss.
  - mat2euler via half-angle atan2: R orthogonal =>
    sqrt(M21^2+M22^2) = sqrt(M00^2+M10^2) = cy = sqrt(1-M20^2), so
      ax = 2*atan(M21/(cy+M22)), ay = -atan(M20*rsqrt(w)),
      az = 2*atan(M10/(cy+M00))
    with one bf16 quake rsqrt + one bundled bf16 quake reciprocal; the ACT
    Arctan LUT is accurate over the full argument range, so no quadrant
    fixes are needed.  Angle scale factors fold into pre-scaled pr targets.
  - per-core output: [128, 4] partial sums; host combines and adds the tiny
    6x6 SVD term (weight2 replicated / computed on host).
"""

import math
import sys

import numpy as np

for _p in ("/opt/trn_rl_repo", "/root/.axon_site/_ro/trn_rl_repo"):
    if _p not in sys.path:
        sys.path.append(_p)

import concourse.bass as bass
from concourse import mybir
from concourse.bass_utils import run_bass_kernel_spmd

B_FULL = 262144
N_CORES = 8
P = 128  # partitions
PI = math.pi

Alu = mybir.AluOpType
Act = mybir.ActivationFunctionType
F32 = mybir.dt.float32
BF16 = mybir.dt.bfloat16

# axis application order for Q = Rx @ Ry @ Rz acting on a column vector:
# z first, then y, then x.  (angle_index, comp_i0, comp_i1, sigma)
# rotation: v_i0' = c*v_i0 + sigma*s*v_i1 ; v_i1' = c*v_i1 - sigma*s*v_i0
AXIS_SPECS = [(2, 0, 1, -1.0), (1, 0, 2, +1.0), (0, 1, 2, -1.0)]


def build_nc(Q=256, nchunk=8, n_iters=1, **kw):
    """Dispatch to the active builder (v2 unless USE_V2 is False)."""
    _V2_KEYS = ("QP", "newton", "skip_term1", "preload", "outlap")
    if USE_V2:
        kw2 = {k: v for k, v in kw.items() if k in _V2_KEYS}
        return build_nc_v2(Q, nchunk, n_iters, **kw2)
    kw1 = {k: v for k, v in kw.items() if k not in _V2_KEYS}
    return build_nc_v1(Q, nchunk, n_iters, **kw1)


def build_nc_v1(Q=256, nchunk=8, n_iters=1, emit_drains=True,
             merge_combine=True, inplace_trig=True, split_dma=True,
             chain_drains=True, fast_recip=False, m2e_drains=True,
             sub_on_dve=False, trig_drains=True, closed_init=True,
             dve_rsqrt=True, pred_swdge=False, dve_tail=True,
             newton_recip=True, chain_steps=8, swap_mult_operands=False,
             skip_term1=False, cce_term1=True):
    """Build the per-core Bass program. Q = items per partition."""
    Bs = P * Q            # items per core
    QC = Q // nchunk      # items/partition per pred chunk
    Q2 = Q // 2

    nc = bass.Bass()

    pred_ext = nc.declare_dram_parameter("pred", [Bs, 9, 6], F32, isOutput=False)
    tar_ext = nc.declare_dram_parameter("tar", [Bs, 54], F32, isOutput=False)
    pr_ext = nc.declare_dram_parameter("pr", [Bs, 6], F32, isOutput=False)
    out_ext = nc.declare_dram_parameter("out", [P, 2], F32, isOutput=True)

    predR = pred_ext.rearrange("(n q) p v -> n q (p v)", n=P)   # [128, Q, 54]
    tarR = tar_ext.rearrange("(n q) v -> n q v", n=P)           # [128, Q, 54]
    prR = pr_ext.rearrange("(n q) v -> n q v", n=P)             # [128, Q, 6]

    # ACT milestones (flag-dependent numbering; every inc is a then_inc)
    n_sins = 8 if split_dma else 2
    A_SINH0 = 4 if split_dma else 2
    A_SIN1 = n_sins
    A_TAU = n_sins + 1
    A_PRB = n_sins + 2
    A_ABS0 = n_sins + 3        # .. (plain mode; cce mode uses maps below)
    if cce_term1:
        # ACT inc order: ..., PRB, neg0, neg1, abs0, neg2, abs1, ..., abs7
        NEG_MS = {}
        ABS_MS = {}
        _c = n_sins + 2
        _order = []
        nxt_neg = 0
        for c in range(nchunk):
            while nxt_neg < min(c + 2, nchunk):
                _c += 1
                NEG_MS[nxt_neg] = _c
                _order.append(("neg", nxt_neg))
                nxt_neg += 1
            _c += 1
            ABS_MS[c] = _c
            _order.append(("abs", c))
        A_ABS_LAST = ABS_MS[nchunk - 1]
        _nxt = _c + 1
    else:
        A_ABS_LAST = A_ABS0 + nchunk - 1
        _nxt = A_ABS0 + nchunk
    if dve_rsqrt:
        A_CY = 0               # no ACT involvement
    else:
        A_CY = _nxt
        _nxt += 1
    if dve_tail:
        A_SG = A_ABSXY = 0
    else:
        A_SG = _nxt
        A_ABSXY = A_SG + 1
        _nxt += 2
    A_ATAN = _nxt
    # VEC milestones (V_W/V_XNEG handshakes exist only for the legacy path)
    if dve_rsqrt and dve_tail:
        V_W = V_XNEG = 0
        V_Z = 1
        V_OUT = 2
    else:
        V_W = 1
        V_XNEG = 2
        V_Z = 3
        V_OUT = 4

    from contextlib import ExitStack
    es = ExitStack()
    with es:
        T = es.enter_context(nc.sbuf_tensor([P, Q, 9, 6], F32))
        if not cce_term1:
            PRED0 = es.enter_context(nc.sbuf_tensor([P, QC, 54], F32))
            PRED1 = es.enter_context(nc.sbuf_tensor([P, QC, 54], F32))
        else:
            PRED0 = PRED1 = None
        PR = es.enter_context(nc.sbuf_tensor([P, Q, 6], F32))
        DFDT = F32 if cce_term1 else BF16
        DF0 = es.enter_context(nc.sbuf_tensor([P, QC, 54], DFDT))
        DF1 = es.enter_context(nc.sbuf_tensor([P, QC, 54], DFDT))
        UH = es.enter_context(nc.sbuf_tensor([P, 9, 3, Q], BF16))
        U4 = es.enter_context(nc.sbuf_tensor([P, 9, 3, Q], BF16))
        # TRIG[0] = cos, TRIG[1] = sin, TRIG[2] = -sin
        TRIG = es.enter_context(nc.sbuf_tensor([P, 3, 9, 3, Q], BF16))
        TAU = es.enter_context(nc.sbuf_tensor([P, 9, 3, Q], BF16))
        PRB = es.enter_context(nc.sbuf_tensor([P, 6, Q], BF16))
        V = es.enter_context(nc.sbuf_tensor([P, 3, 3, Q], BF16))
        U = es.enter_context(nc.sbuf_tensor([P, 2, 3, 2, Q], BF16))
        XY = es.enter_context(nc.sbuf_tensor([P, 2, 3, Q], F32))
        SCR = es.enter_context(nc.sbuf_tensor([P, 8, 3, Q], F32))
        MSK = es.enter_context(nc.sbuf_tensor([P, 2, 3, Q], mybir.dt.uint8))
        TPB = es.enter_context(nc.sbuf_tensor([P, 3, Q], BF16))
        SGB = es.enter_context(nc.sbuf_tensor([P, 3, Q], BF16))
        TMPB = es.enter_context(nc.sbuf_tensor([P, 3, Q], BF16))
        D2B = es.enter_context(nc.sbuf_tensor([P, 2, 3, Q], BF16))
        A1 = es.enter_context(nc.sbuf_tensor([P, nchunk], F32))
        OUT = es.enter_context(nc.sbuf_tensor([P, 2], F32))
        dma_t = nc.alloc_semaphore("dma_t")
        dma_tb = nc.alloc_semaphore("dma_tb")
        dma_t2 = nc.alloc_semaphore("dma_t2")
        dma_td = nc.alloc_semaphore("dma_td")
        dma_p0 = nc.alloc_semaphore("dma_p0")
        dma_p1 = nc.alloc_semaphore("dma_p1")
        dma_pr = nc.alloc_semaphore("dma_pr")
        dma_o = nc.alloc_semaphore("dma_o")
        s_act = nc.alloc_semaphore("s_act")
        s_vec = nc.alloc_semaphore("s_vec")
        s_gp = nc.alloc_semaphore("s_gp")
        all_sems = [dma_t, dma_tb, dma_t2, dma_td, dma_p0, dma_p1, dma_pr,
                    dma_o, s_act, s_vec, s_gp]
        for _it in range(n_iters):
            it_es = ExitStack()
            with it_es:
                block = it_es.enter_context(nc.Block())
                PREDb = [PRED0, PRED1]
                DFb = [DF0, DF1]

                @block.sync
                def _(sync):
                    if split_dma:
                        Q4 = Q // 4
                        sync.dma_start(
                            out=T[:, 0:Q4, :, :], in_=tarR[:, 0:Q4, :]
                        ).then_inc(dma_t, 16)
                        sync.dma_start(
                            out=T[:, Q4:Q2, :, :], in_=tarR[:, Q4:Q2, :]
                        ).then_inc(dma_tb, 16)
                    else:
                        sync.dma_start(out=T[:], in_=tarR).then_inc(dma_t, 16)
                    sync.dma_start(out=PR[:], in_=prR).then_inc(dma_pr, 16)
                    if not pred_swdge and not skip_term1 and not cce_term1:
                        for c in range(nchunk):
                            if c >= 2:
                                # WAR: gpsimd must be done with PREDb[c % 2]
                                sync.wait_ge(s_gp, c - 1)
                            sync.dma_start(
                                out=PREDb[c % 2][:],
                                in_=predR[:, c * QC:(c + 1) * QC, :],
                            ).then_inc([dma_p0, dma_p1][c % 2], 16)
                    sync.wait_ge(s_vec, V_OUT)
                    sync.dma_start(out=out_ext[:], in_=OUT[:]).then_inc(dma_o, 16)
                    sync.wait_ge(dma_o, 16)

                @block.gpsimd
                def _(gpsimd):
                    if cce_term1 and not skip_term1:
                        for c in range(nchunk):
                            # DF[c%2] holds -tar chunk once ACT's neg_c ran
                            gpsimd.wait_ge(s_act, NEG_MS[c])
                            gpsimd.dma_start(
                                out=DFb[c % 2][:],
                                in_=predR[:, c * QC:(c + 1) * QC, :],
                                accum_op=Alu.add,
                            ).then_inc([dma_p0, dma_p1][c % 2], 16)
                    if skip_term1:
                        gpsimd.sem_inc(s_gp, nchunk)
                    if pred_swdge and not sub_on_dve and not skip_term1:
                        for c in range(min(2, nchunk)):
                            gpsimd.dma_start(
                                out=PREDb[c % 2][:],
                                in_=predR[:, c * QC:(c + 1) * QC, :],
                            ).then_inc([dma_p0, dma_p1][c % 2], 16)
                    for c in range(nchunk if not (sub_on_dve or skip_term1
                                                   or cce_term1) else 0):
                        if split_dma:
                            Q4 = Q // 4
                            hi_q = (c + 1) * QC
                            qsem = (dma_t if hi_q <= Q4 else
                                    dma_tb if hi_q <= Q2 else
                                    dma_t2 if hi_q <= Q2 + Q4 else dma_td)
                            gpsimd.wait_ge(qsem, 16)
                        else:
                            gpsimd.wait_ge(dma_t, 16)
                        gpsimd.wait_ge([dma_p0, dma_p1][c % 2], 16 * (c // 2 + 1))
                        if c >= 2:
                            # WAR: ACT must be done abs-ing DFb[c % 2]
                            gpsimd.wait_ge(s_act, A_ABS0 + (c - 2))

                        gpsimd.tensor_tensor(
                            out=DFb[c % 2][:],
                            in0=PREDb[c % 2][:],
                            in1=T[:, c * QC:(c + 1) * QC, :, :].rearrange(
                                "n q p v -> n q (p v)"
                            ),
                            op=Alu.subtract,
                        ).then_inc(s_gp, 1)
                        if pred_swdge and c + 2 < nchunk:
                            # prefetch chunk c+2 into the buffer just freed
                            gpsimd.wait_ge(s_gp, c + 1)
                            gpsimd.dma_start(
                                out=PREDb[c % 2][:],
                                in_=predR[:, (c + 2) * QC:(c + 3) * QC, :],
                            ).then_inc([dma_p0, dma_p1][c % 2], 16)

                @block.scalar
                def _(scalar):
                    act = nc.scalar
                    if split_dma:
                        Q4 = Q // 4
                        scalar.dma_start(
                            out=T[:, Q2:Q2 + Q4, :, :], in_=tarR[:, Q2:Q2 + Q4, :]
                        ).then_inc(dma_t2, 16)
                        scalar.dma_start(
                            out=T[:, Q2 + Q4:, :, :], in_=tarR[:, Q2 + Q4:, :]
                        ).then_inc(dma_td, 16)
                        quarters = [
                            (0, Q4, dma_t, 16), (Q4, Q2, dma_tb, 16),
                            (Q2, Q2 + Q4, dma_t2, 16), (Q2 + Q4, Q, dma_td, 16),
                        ]
                    else:
                        quarters = [(0, Q, dma_t, 16)]
                    for (lo, hi, sem, thr) in quarters:
                        scalar.wait_ge(sem, thr)
                        ang = T[:, lo:hi, :, 3:6].transpose([0, 2, 3, 1])
                        act.activation(
                            UH[:, :, :, lo:hi], ang, Act.Sin, scale=0.5
                        ).then_inc(s_act, 1)
                        act.activation(
                            U4[:, :, :, lo:hi], ang, Act.Sin, scale=0.25
                        ).then_inc(s_act, 1)
                    if split_dma:
                        scalar.wait_ge(dma_tb, 16)
                        scalar.wait_ge(dma_td, 16)
                    taus = T[:, :, :, 0:3].transpose([0, 2, 3, 1])
                    act.activation(TAU[:], taus, Act.Copy).then_inc(s_act, 1)
                    scalar.wait_ge(dma_pr, 16)
                    act.activation(
                        PRB[:], PR[:].transpose([0, 2, 1]), Act.Copy
                    ).then_inc(s_act, 1)
                    if skip_term1:
                        scalar.sem_inc(s_act, 2 * nchunk if cce_term1 else nchunk)
                    elif cce_term1:
                        Q4 = Q // 4
                        def tchunk_sem(c):
                            hi_q = (c + 1) * QC
                            return (dma_t if hi_q <= Q4 else
                                    dma_tb if hi_q <= Q2 else
                                    dma_t2 if hi_q <= Q2 + Q4 else dma_td)
                        for (kind, c) in _order:
                            if kind == "neg":
                                scalar.wait_ge(tchunk_sem(c), 16)
                                if c >= 2:
                                    # same-engine WAW on DF[c%2] vs abs_{c-2}
                                    scalar.drain()
                                act.activation(
                                    DFb[c % 2][:],
                                    T[:, c * QC:(c + 1) * QC, :, :].rearrange(
                                        "n q p v -> n q (p v)"),
                                    Act.Copy, scale=-1.0,
                                ).then_inc(s_act, 1)
                            else:
                                scalar.wait_ge(
                                    [dma_p0, dma_p1][c % 2], 16 * (c // 2 + 1)
                                )
                                act.activation(
                                    DFb[c % 2][:], DFb[c % 2][:], Act.Abs,
                                    accum_out=A1[:, c:c + 1],
                                ).then_inc(s_act, 1)
                    else:
                        for c in range(nchunk):
                            scalar.wait_ge(s_gp, c + 1)
                            act.activation(
                                DFb[c % 2][:], DFb[c % 2][:], Act.Abs,
                                accum_out=A1[:, c:c + 1],
                            ).then_inc(s_act, 1)
                    # mat2euler support
                    if not (dve_rsqrt and dve_tail):
                        # (redundant when both: the later V_Z wait dominates)
                        scalar.wait_ge(s_vec, V_W)
                    if not dve_rsqrt:
                        act.activation(
                            XY[:, 1, 1, :], SCR[:, 7, 1, :], Act.Sqrt
                        ).then_inc(s_act, 1)  # cy
                    if not dve_tail:
                        act.activation(
                            SCR[:, 4, :, :], XY[:, 0, :, :], Act.Sign
                        ).then_inc(s_act, 1)  # sign(Y)
                        scalar.wait_ge(s_vec, V_XNEG)
                        scalar.drain()
                        act.activation(XY[:], XY[:], Act.Abs).then_inc(s_act, 1)
                    scalar.wait_ge(s_vec, V_Z)
                    act.activation(
                        TPB[:], SCR[:, 0, :, :], Act.Arctan
                    ).then_inc(s_act, 1)

                @block.vector
                def _(vector):
                    vec = nc.vector

                    def dv():
                        if emit_drains:
                            vector.drain()

                    if sub_on_dve:
                        for c in range(nchunk):
                            Q4 = Q // 4
                            hi_q = (c + 1) * QC
                            if split_dma:
                                qsem = (dma_t if hi_q <= Q4 else
                                        dma_tb if hi_q <= Q2 else
                                        dma_t2 if hi_q <= Q2 + Q4 else dma_td)
                                vector.wait_ge(qsem, 16)
                            else:
                                vector.wait_ge(dma_t, 16)
                            vector.wait_ge(
                                [dma_p0, dma_p1][c % 2], 16 * (c // 2 + 1)
                            )
                            if c >= 2:
                                vector.wait_ge(s_act, A_ABS0 + (c - 2))
                            vec.tensor_tensor(
                                out=DFb[c % 2][:],
                                in0=PREDb[c % 2][:],
                                in1=T[:, c * QC:(c + 1) * QC, :, :].rearrange(
                                    "n q p v -> n q (p v)"
                                ),
                                op=Alu.subtract,
                            ).then_inc(s_gp, 1)
                            dv()

                    def tdv():
                        if emit_drains and trig_drains:
                            vector.drain()

                    if inplace_trig:
                        for (lo, hi, thr) in (
                            (0, Q2, A_SINH0), (Q2, Q, A_SIN1)
                        ) if split_dma else ((0, Q, A_SIN1),):
                            vector.wait_ge(s_act, thr)
                            u4s = U4[:, :, :, lo:hi]
                            uhs = UH[:, :, :, lo:hi]
                            # U4 <- cos(x/2) = 1 - 2*sin(x/4)^2   (in place)
                            vec.tensor_tensor(
                                out=u4s, in0=u4s, in1=u4s, op=Alu.mult
                            )
                            tdv()
                            vec.tensor_scalar(
                                u4s, u4s, -2.0, 1.0, Alu.mult, Alu.add
                            )
                            tdv()
                            # S = sin(x) = 2*sin(x/2)*cos(x/2)
                            vec.tensor_tensor(
                                out=TRIG[:, 1, :, :, lo:hi], in0=uhs, in1=u4s,
                                op=Alu.mult,
                            )
                            tdv()
                            vec.tensor_scalar(
                                TRIG[:, 1, :, :, lo:hi],
                                TRIG[:, 1, :, :, lo:hi], 2.0, None, Alu.mult
                            )
                            tdv()
                            vec.tensor_tensor(
                                out=uhs, in0=uhs, in1=uhs, op=Alu.mult
                            )
                            tdv()
                            vec.tensor_scalar(
                                TRIG[:, 0, :, :, lo:hi], uhs, -2.0, 1.0,
                                Alu.mult, Alu.add
                            )
                            tdv()
                    else:
                        vector.wait_ge(s_act, A_SIN1)
                        # STT variants (no in-place aliasing)
                        vec.scalar_tensor_tensor(
                            out=U4[:], in0=U4[:], scalar=-2.0, in1=U4[:],
                            op0=Alu.mult, op1=Alu.mult,
                        )
                        tdv()
                        vec.tensor_scalar(U4[:], U4[:], 1.0, None, Alu.add)
                        tdv()
                        vec.scalar_tensor_tensor(
                            out=TRIG[:, 1], in0=UH[:], scalar=2.0, in1=U4[:],
                            op0=Alu.mult, op1=Alu.mult,
                        )
                        tdv()
                        vec.scalar_tensor_tensor(
                            out=TRIG[:, 0], in0=UH[:], scalar=-2.0, in1=UH[:],
                            op0=Alu.mult, op1=Alu.mult,
                        )
                        tdv()
                        vec.tensor_scalar(
                            TRIG[:, 0], TRIG[:, 0], 1.0, None, Alu.add
                        )
                        tdv()
                    vec.tensor_scalar(TRIG[:, 2], TRIG[:, 1], -1.0, None, Alu.mult)
                    tdv()
                    def cdv():
                        if emit_drains and chain_drains:
                            vector.drain()

                    vector.wait_ge(s_act, A_TAU)
                    if closed_init:
                        # V <- (col0(Q9), col1(Q9), tau9) in closed form
                        C9 = lambda a: TRIG[:, 0, 8, a, :]
                        S9 = lambda a: TRIG[:, 1, 8, a, :]
                        u = lambda m, v, p: U[:, m, v, p, :]
                        vec.tensor_tensor(out=u(0, 0, 0), in0=S9(0), in1=S9(1),
                                          op=Alu.mult)  # sxsy
                        vec.tensor_tensor(out=u(0, 0, 1), in0=C9(0), in1=S9(1),
                                          op=Alu.mult)  # cxsy
                        vec.tensor_tensor(out=V[:, 0, 0, :], in0=C9(1),
                                          in1=C9(2), op=Alu.mult)  # cy*cz
                        vec.tensor_tensor(out=u(0, 1, 0), in0=C9(0), in1=S9(2),
                                          op=Alu.mult)  # cx*sz
                        vec.tensor_tensor(out=u(1, 0, 0), in0=C9(1), in1=S9(2),
                                          op=Alu.mult)  # cy*sz
                        vec.tensor_tensor(out=u(0, 2, 0), in0=S9(0), in1=S9(2),
                                          op=Alu.mult)  # sx*sz
                        vec.tensor_tensor(out=u(1, 0, 1), in0=C9(0), in1=C9(2),
                                          op=Alu.mult)  # cx*cz
                        vec.tensor_tensor(out=u(1, 1, 1), in0=S9(0), in1=C9(2),
                                          op=Alu.mult)  # sx*cz
                        cdv()
                        vec.tensor_tensor(out=u(0, 1, 1), in0=u(0, 0, 0),
                                          in1=C9(2), op=Alu.mult)  # sxsy*cz
                        vec.tensor_tensor(out=u(0, 2, 1), in0=u(0, 0, 1),
                                          in1=C9(2), op=Alu.mult)  # cxsy*cz
                        vec.tensor_tensor(out=u(1, 1, 0), in0=u(0, 0, 0),
                                          in1=S9(2), op=Alu.mult)  # sxsy*sz
                        vec.tensor_tensor(out=u(1, 2, 0), in0=u(0, 0, 1),
                                          in1=S9(2), op=Alu.mult)  # cxsy*sz
                        vec.tensor_scalar_mul(V[:, 1, 0, :], u(1, 0, 0), -1.0)
                        cdv()
                        vec.tensor_tensor(out=V[:, 0, 1, :], in0=u(0, 1, 0),
                                          in1=u(0, 1, 1), op=Alu.add)
                        vec.tensor_tensor(out=V[:, 0, 2, :], in0=u(0, 2, 0),
                                          in1=u(0, 2, 1), op=Alu.subtract)
                        vec.tensor_tensor(out=V[:, 1, 1, :], in0=u(1, 0, 1),
                                          in1=u(1, 1, 0), op=Alu.subtract)
                        vec.tensor_tensor(out=V[:, 1, 2, :], in0=u(1, 1, 1),
                                          in1=u(1, 2, 0), op=Alu.add)
                        vec.tensor_copy(out=V[:, 2, :, :], in_=TAU[:, 8])
                        cdv()
                        k_range = range(chain_steps, 0, -1)
                    else:
                        vec.memset(V[:], 0.0)
                        dv()
                        vec.memset(V[:, 0, 0, :], 1.0)
                        dv()
                        vec.memset(V[:, 1, 1, :], 1.0)
                        dv()
                        k_range = range(chain_steps + 1, 0, -1)

                    for k in k_range:               # descending pose index
                        pk = k - 1
                        for (a, i0, i1, sigma) in AXIS_SPECS:
                            d = i1 - i0
                            pair = V[:, :, i0:i1 + 1:d, :]
                            # c-products: U[0][v][p] = c * v_pair[p]
                            cb = (TRIG[:, 0, pk, a, :].unsqueeze(1)
                                  .unsqueeze(2).broadcast_to([P, 3, 2, Q]))
                            if swap_mult_operands:
                                vec.tensor_tensor(out=U[:, 0], in0=pair,
                                                  in1=cb, op=Alu.mult)
                            else:
                                vec.tensor_tensor(out=U[:, 0], in0=cb,
                                                  in1=pair, op=Alu.mult)
                            # s-products with per-slot sign, reversed pair:
                            #   sigma=-1: U[1][v] = (-s*v_i1, +s*v_i0)
                            #   sigma=+1: U[1][v] = (+s*v_i1, -s*v_i0)
                            if merge_combine:
                                strig = (TRIG[:, 2:0:-1, pk, a, :] if sigma < 0
                                         else TRIG[:, 1:3, pk, a, :])
                                rpair = (V[:, :, 1::-1, :] if (i0, i1) == (0, 1)
                                         else V[:, :, 2::-2, :]
                                         if (i0, i1) == (0, 2)
                                         else V[:, :, 2:0:-1, :])
                                sb = strig.unsqueeze(1).broadcast_to(
                                    [P, 3, 2, Q])
                                if swap_mult_operands:
                                    vec.tensor_tensor(out=U[:, 1], in0=rpair,
                                                      in1=sb, op=Alu.mult)
                                else:
                                    vec.tensor_tensor(out=U[:, 1], in0=sb,
                                                      in1=rpair, op=Alu.mult)
                                cdv()
                                vec.tensor_tensor(
                                    out=pair, in0=U[:, 0], in1=U[:, 1],
                                    op=Alu.add,
                                )
                                cdv()
                            else:
                                vec.tensor_tensor(
                                    out=U[:, 1], in1=pair, op=Alu.mult,
                                    in0=TRIG[:, 1, pk, a, :].unsqueeze(1)
                                    .unsqueeze(2).broadcast_to([P, 3, 2, Q]),
                                )
                                cdv()
                                vec.tensor_tensor(
                                    out=V[:, :, i0, :],
                                    in0=U[:, 0, :, 0, :], in1=U[:, 1, :, 1, :],
                                    op=Alu.add if sigma > 0 else Alu.subtract,
                                )
                                vec.tensor_tensor(
                                    out=V[:, :, i1, :],
                                    in0=U[:, 0, :, 1, :], in1=U[:, 1, :, 0, :],
                                    op=Alu.subtract if sigma > 0 else Alu.add,
                                )
                                cdv()
                        # t += tau_k
                        vec.tensor_tensor(
                            out=V[:, 2, :, :], in0=V[:, 2, :, :], in1=TAU[:, pk],
                            op=Alu.add,
                        )
                        cdv()
                    def mdv():
                        if emit_drains and m2e_drains:
                            vector.drain()

                    # mat2euler inputs
                    # M22 = M00*M11 - M10*M01
                    vec.tensor_tensor(
                        out=XY[:, 1, 0, :], in0=V[:, 0, 0, :], in1=V[:, 1, 1, :],
                        op=Alu.mult,
                    )
                    vec.tensor_tensor(
                        out=SCR[:, 7, 0, :], in0=V[:, 0, 1, :], in1=V[:, 1, 0, :],
                        op=Alu.mult,
                    )
                    mdv()
                    vec.tensor_tensor(
                        out=XY[:, 1, 0, :], in0=XY[:, 1, 0, :],
                        in1=SCR[:, 7, 0, :], op=Alu.subtract,
                    )
                    # w = M00^2 + M10^2
                    vec.tensor_tensor(
                        out=SCR[:, 7, 1, :], in0=V[:, 0, 0, :], in1=V[:, 0, 0, :],
                        op=Alu.mult,
                    )
                    vec.tensor_tensor(
                        out=SCR[:, 7, 2, :], in0=V[:, 0, 1, :], in1=V[:, 0, 1, :],
                        op=Alu.mult,
                    )
                    mdv()
                    vec.tensor_tensor(
                        out=SCR[:, 7, 1, :], in0=SCR[:, 7, 1, :],
                        in1=SCR[:, 7, 2, :], op=Alu.add,
                    )
                    # Y = (M21, -M20, M10); X = (M22, cy, M00)
                    vec.tensor_copy(out=XY[:, 0, 0, :], in_=V[:, 1, 2, :])
                    vec.tensor_scalar_mul(XY[:, 0, 1, :], V[:, 0, 2, :], -1.0)
                    vec.tensor_copy(out=XY[:, 0, 2, :], in_=V[:, 0, 1, :])
                    _i = vec.tensor_copy(out=XY[:, 1, 2, :], in_=V[:, 0, 0, :])
                    if V_W:
                        _i.then_inc(s_vec, 1)  # V_W
                    if dve_rsqrt:
                        # cy = w * rsqrt(w): quake seed + 2 Newton iterations
                        w_ap = SCR[:, 7, 1, :]
                        t0 = SCR[:, 7, 0, :]
                        r_ap = SCR[:, 7, 2, :]
                        mdv()
                        vec.tensor_scalar(
                            t0.bitcast(mybir.dt.int32),
                            w_ap.bitcast(mybir.dt.int32), 1, None,
                            Alu.logical_shift_right,
                        )
                        mdv()
                        vec.tensor_scalar(
                            r_ap.bitcast(mybir.dt.int32),
                            t0.bitcast(mybir.dt.int32), -1, 0x5F3759DF,
                            Alu.mult, Alu.add,
                        )
                        for _nit in range(2):
                            mdv()
                            vec.tensor_tensor(out=t0, in0=r_ap, in1=r_ap,
                                              op=Alu.mult)
                            mdv()
                            vec.tensor_tensor(out=t0, in0=t0, in1=w_ap,
                                              op=Alu.mult)
                            mdv()
                            vec.tensor_scalar(t0, t0, -0.5, 1.5,
                                              Alu.mult, Alu.add)
                            mdv()
                            vec.tensor_tensor(out=r_ap, in0=r_ap, in1=t0,
                                              op=Alu.mult)
                        mdv()
                        vec.tensor_tensor(out=XY[:, 1, 1, :], in0=w_ap,
                                          in1=r_ap, op=Alu.mult)
                    if not dve_rsqrt:
                        vector.wait_ge(s_act, A_CY)
                    mdv()
                    _i = vec.tensor_scalar(
                        MSK[:, 1, :, :], XY[:, 1, :, :], 0.0, None, Alu.is_lt
                    )
                    if V_XNEG:
                        _i.then_inc(s_vec, 1)  # V_XNEG
                    if dve_tail:
                        # sg = 2*(y >= 0) - 1
                        vec.tensor_scalar(
                            SGB[:], XY[:, 0, :, :], 0.0, None, Alu.is_ge,
                        )
                        mdv()
                        vec.tensor_scalar(
                            SGB[:], SGB[:], 2.0, -1.0, Alu.mult, Alu.add,
                        )
                        # |XY| in place via sign-bit clear
                        vec.tensor_scalar(
                            XY[:].bitcast(mybir.dt.int32),
                            XY[:].bitcast(mybir.dt.int32),
                            0x7FFFFFFF, None, Alu.bitwise_and,
                        )
                    else:
                        vector.wait_ge(s_act, A_ABSXY)
                    mdv()
                    vec.tensor_tensor(
                        out=SCR[:, 0, :, :], in0=XY[:, 0, :, :],
                        in1=XY[:, 1, :, :], op=Alu.min,
                    )
                    vec.tensor_tensor(
                        out=SCR[:, 1, :, :], in0=XY[:, 0, :, :],
                        in1=XY[:, 1, :, :], op=Alu.max,
                    )
                    vec.tensor_tensor(
                        out=MSK[:, 0, :, :], in0=XY[:, 0, :, :],
                        in1=XY[:, 1, :, :], op=Alu.is_gt,
                    )
                    mdv()
                    if newton_recip:
                        mx = SCR[:, 1, :, :]
                        rc = SCR[:, 2, :, :]
                        t7 = SCR[:, 7, :, :]
                        vec.tensor_scalar(
                            rc.bitcast(mybir.dt.int32),
                            mx.bitcast(mybir.dt.int32), -1, 0x7EF477D5,
                            Alu.mult, Alu.add,
                        )
                        for _nit in range(2):
                            mdv()
                            vec.tensor_tensor(out=t7, in0=mx, in1=rc,
                                              op=Alu.mult)
                            mdv()
                            vec.tensor_scalar(t7, t7, -1.0, 2.0,
                                              Alu.mult, Alu.add)
                            mdv()
                            vec.tensor_tensor(out=rc, in0=rc, in1=t7,
                                              op=Alu.mult)
                    elif fast_recip:
                        vec.reciprocal_approx_fast(
                            out=SCR[:, 2, :, :], in_=SCR[:, 1, :, :]
                        )
                    else:
                        vec.reciprocal(out=SCR[:, 2, :, :], in_=SCR[:, 1, :, :])
                    mdv()
                    vec.tensor_tensor(
                        out=SCR[:, 0, :, :], in0=SCR[:, 0, :, :],
                        in1=SCR[:, 2, :, :], op=Alu.mult,
                    ).then_inc(s_vec, 1)  # z = mn/mx -> V_Z
                    vector.wait_ge(s_act, A_ATAN)
                    # swap fix: t <- pi/2 - t where |y| > |x|
                    vec.tensor_scalar(
                        TMPB[:], TPB[:], -1.0, PI / 2, Alu.mult, Alu.add,
                    )
                    mdv()
                    vec.copy_predicated(
                        out=TPB[:], mask=MSK[:, 0, :, :], data=TMPB[:],
                    )
                    mdv()
                    # quadrant fix: t <- pi - t where x < 0
                    vec.tensor_scalar(
                        TMPB[:], TPB[:], -1.0, PI, Alu.mult, Alu.add,
                    )
                    mdv()
                    vec.copy_predicated(
                        out=TPB[:], mask=MSK[:, 1, :, :], data=TMPB[:],
                    )
                    mdv()
                    # apply sign(y)
                    vec.tensor_tensor(
                        out=TPB[:], in0=TPB[:], in1=SGB[:], op=Alu.mult,
                    )
                    # term2 diffs vs pr_glpose
                    vector.wait_ge(s_act, A_PRB)
                    vec.tensor_tensor(
                        out=D2B[:, 0], in0=V[:, 2, :, :],
                        in1=PRB[:, 0:3, :], op=Alu.subtract,
                    )
                    mdv()
                    vec.tensor_tensor(
                        out=D2B[:, 1], in0=TPB[:],
                        in1=PRB[:, 3:6, :], op=Alu.subtract,
                    )
                    mdv()
                    vec.tensor_reduce(
                        out=OUT[:, 1:2], in_=D2B[:],
                        axis=mybir.AxisListType.XYZ,
                        op=Alu.add, apply_absolute_value=True,
                    )
                    vector.wait_ge(s_act, A_ABS_LAST)
                    mdv()
                    vec.tensor_reduce(
                        out=OUT[:, 0:1], in_=A1[:], axis=mybir.AxisListType.X,
                        op=Alu.add,
                    ).then_inc(s_vec, 1)  # V_OUT

            used = [nc.sync.engine, nc.gpsimd.engine, nc.scalar.engine,
                    nc.vector.engine]
            nc.multi_engine_barrier(used)
            import itertools
            nums = sorted(s.num for s in all_sems)
            for _, grp in itertools.groupby(
                enumerate(nums), lambda t: t[1] - t[0]
            ):
                g = [n for _, n in grp]
                rng = range(g[0], g[-1] + 1)
                nc.gpsimd.dma_reset(rng)
                nc.gpsimd.sem_clear(rng)
            nc.multi_engine_barrier(used)

    return nc


def build_nc_v2(Q=256, nchunk=8, n_iters=1, QP=56, skip_term1=False,
                newton=1, preload=0, outlap=0, **_ignored):
    """v2 pipeline.

    - term1: tar streams into T; after ACT consumes a quarter's angles +
      translations, a gpsimd CCE DMA overwrites that T chunk with pred-tar
      in place; ACT Abs(+accum) reduces it.  No PRED buffers, no neg pass.
    - trig: ACT Sin(x/2)->H, Sin(x/4)->QT only.  DVE: qs=QT*QT (in place),
      g'=-4*qs+2 (=2cos(x/2)), hs=H*H -> TRIG0, C=-2*hs+1 (in place),
      S=H*g' -> TRIG1, negS=-S -> TRIG2.  No STT (runs 1x), TS ops hit 4x.
    - chain: 9-step descending Givens chain on V=(col0,col1,tau).  QP>0
      would split [0:QP] onto gpsimd (Pool), but ANY Pool compute measured
      ~+40us/iter on HW (shared DVE/Pool SBUF port) — use QP=0.
    - mat2euler tail via half-angle atan2: because R is orthogonal,
      sqrt(M21^2+M22^2) = sqrt(M00^2+M10^2) = cy = sqrt(1-M20^2), so
        ax = 2*atan(M21/(cy+M22)), ay = -atan(M20/cy), az = 2*atan(M10/(cy+M00))
      with bf16 int16-bitcast quake rsqrt/recip (`newton` extra iterations;
      seed-only keeps rel err ~4e-4).  No quadrant fixes / masks / predicated
      copies (Arctan LUT verified accurate to |x|~4e4 on HW).
    """
    Bs = P * Q
    QC = Q // nchunk
    Q4 = Q // 4
    assert QP <= Q4, "Pool chain range must sit inside quarter 0"

    nc = bass.Bass()

    pred_ext = nc.declare_dram_parameter("pred", [Bs, 9, 6], F32, isOutput=False)
    tar_ext = nc.declare_dram_parameter("tar", [Bs, 54], F32, isOutput=False)
    pr_ext = nc.declare_dram_parameter("pr", [Bs, 6], F32, isOutput=False)
    out_ext = nc.declare_dram_parameter("out", [P, 4], F32, isOutput=True)

    predR = pred_ext.rearrange("(n q) p v -> n q (p v)", n=P)   # [128, Q, 54]
    tarR = tar_ext.rearrange("(n q) v -> n q v", n=P)           # [128, Q, 54]
    prR = pr_ext.rearrange("(n q) v -> n q v", n=P)             # [128, Q, 6]

    # ACT milestones (then_inc 1 each, cumulative order):
    #   sins per quarter q: qsin=2q+1, h=2q+2 (q=0..3); taus 9..12; PRB=13;
    #   abs_c=14+c; atan last.  Taus go after all sins so ACT never delays
    #   the q3 sins that gate the chain start.
    MS_QSIN = lambda q: 2 * q + 1
    MS_SIN = lambda q: 2 * q + 2
    MS_TAU = lambda q: 9 + q
    MS_PRB = 15          # after 3 PRB prep copies (13, 14, 15)
    MS_ABS = lambda c: 16 + c
    MS_ATAN = 16 + nchunk
    MS_OUT = MS_ATAN + 4
    # DVE milestones: trig quarter q done = q+1; Z, then the two diff vectors
    V_TRIG = lambda q: q + 1
    V_Z = 5
    V_D0 = 6
    V_D1 = 7
    # Pool milestone: chain-P done = 1
    GP_CHAIN = 1

    from contextlib import ExitStack
    es = ExitStack()
    with es:
        T = es.enter_context(nc.sbuf_tensor([P, Q, 9, 6], F32))
        PR = es.enter_context(nc.sbuf_tensor([P, Q, 6], F32))
        PRB = es.enter_context(nc.sbuf_tensor([P, 6, Q], BF16))
        H = es.enter_context(nc.sbuf_tensor([P, 9, 3, Q], BF16))
        QT = es.enter_context(nc.sbuf_tensor([P, 9, 3, Q], BF16))
        # TRIG[0]=C, TRIG[1]=S, TRIG[2]=-S
        TRIG = es.enter_context(nc.sbuf_tensor([P, 3, 9, 3, Q], BF16))
        TAU = es.enter_context(nc.sbuf_tensor([P, 9, 3, Q], BF16))
        V = es.enter_context(nc.sbuf_tensor([P, 3, 3, Q], BF16))
        U = es.enter_context(nc.sbuf_tensor([P, 2, 3, 2, Q], BF16))
        # tail scratch
        WB = es.enter_context(nc.sbuf_tensor([P, 3, Q], BF16))   # w,t,u
        M2B = es.enter_context(nc.sbuf_tensor([P, 3, Q], BF16))  # x1,x2,M22
        DEN = es.enter_context(nc.sbuf_tensor([P, 2, Q], BF16))
        RC = es.enter_context(nc.sbuf_tensor([P, 2, Q], BF16))
        RT = es.enter_context(nc.sbuf_tensor([P, 2, Q], BF16))
        CY = es.enter_context(nc.sbuf_tensor([P, 1, Q], BF16))
        ZB = es.enter_context(nc.sbuf_tensor([P, 3, Q], BF16))
        TPB = es.enter_context(nc.sbuf_tensor([P, 3, Q], BF16))
        D2B = es.enter_context(nc.sbuf_tensor([P, 2, 3, Q], BF16))
        A1 = es.enter_context(nc.sbuf_tensor([P, nchunk], F32))
        OUT = es.enter_context(nc.sbuf_tensor([P, 4], F32))

        dma_t = [nc.alloc_semaphore(f"dma_t{i}") for i in range(4)]
        dma_pr = nc.alloc_semaphore("dma_pr")
        dma_pc = nc.alloc_semaphore("dma_pc")
        dma_o = nc.alloc_semaphore("dma_o")
        s_act = nc.alloc_semaphore("s_act")
        s_vec = nc.alloc_semaphore("s_vec")
        s_gp = nc.alloc_semaphore("s_gp")
        all_sems = dma_t + [dma_pr, dma_pc, dma_o, s_act, s_vec, s_gp]

        def qrange(q):
            return q * Q4, (q + 1) * Q4

        for _it in range(n_iters):
            it_es = ExitStack()
            with it_es:
                block = it_es.enter_context(nc.Block())

                @block.sync
                def _(sync):
                    it = _it

                    def issue_inputs():
                        for q in range(4):
                            lo, hi = qrange(q)
                            sync.dma_start(
                                out=T[:, lo:hi, :, :], in_=tarR[:, lo:hi, :]
                            ).then_inc(dma_t[q], 16)
                        sync.dma_start(out=PR[:], in_=prR).then_inc(dma_pr, 16)

                    if not preload or it == 0:
                        issue_inputs()
                    if preload and it < n_iters - 1:
                        # prefetch next iteration's inputs while this
                        # iteration's chain runs: T quarter q is dead after
                        # abs of chunks 2q/2q+1; PR after the PRB copies.
                        for q in range(4):
                            lo, hi = qrange(q)
                            sync.wait_ge(s_act, MS_ABS(2 * q + 1))
                            sync.dma_start(
                                out=T[:, lo:hi, :, :], in_=tarR[:, lo:hi, :]
                            ).then_inc(dma_t[q], 16)
                        sync.dma_start(out=PR[:], in_=prR).then_inc(dma_pr, 16)
                    sync.wait_ge(s_act, MS_OUT)
                    sync.dma_start(out=out_ext[:], in_=OUT[:]).then_inc(dma_o, 16)
                    if outlap:
                        # monotonic dma_o: only the final iteration must wait
                        # for its OUT to land; earlier OUTs are overwritten.
                        if it == n_iters - 1:
                            sync.wait_ge(dma_o, 16 * n_iters)
                    else:
                        sync.wait_ge(dma_o, 16)

                @block.scalar
                def _(scalar):
                    act = nc.scalar
                    dthr = 16 * (_it + 1) if preload else 16
                    # T holds -tar (host negates); flip ACT scale signs.
                    # quarter-sin first: DVE's first trig op needs only QT.
                    for q in range(4):
                        lo, hi = qrange(q)
                        scalar.wait_ge(dma_t[q], dthr)
                        ang = T[:, lo:hi, :, 3:6].transpose([0, 2, 3, 1])
                        act.activation(
                            QT[:, :, :, lo:hi], ang, Act.Sin, scale=-0.25
                        ).then_inc(s_act, 1)
                        act.activation(
                            H[:, :, :, lo:hi], ang, Act.Sin, scale=-0.5
                        ).then_inc(s_act, 1)
                    for q in range(4):
                        lo, hi = qrange(q)
                        taus = T[:, lo:hi, :, 0:3].transpose([0, 2, 3, 1])
                        act.activation(
                            TAU[:, :, :, lo:hi], taus, Act.Copy, scale=-1.0
                        ).then_inc(s_act, 1)
                    scalar.wait_ge(dma_pr, dthr)
                    # PRB = [pr0,pr1,pr2, pr3/2, -pr4, pr5/2]: the /2 folds the
                    # half-angle doubling (ax=2t0, az=2t2) into the reference;
                    # -pr4 folds ay=-t1.  Host scales col2 sums by 2.
                    act.activation(
                        PRB[:, 0:3, :], PR[:, :, 0:3].transpose([0, 2, 1]),
                        Act.Copy,
                    ).then_inc(s_act, 1)
                    act.activation(
                        PRB[:, 3:6:2, :], PR[:, :, 3:6:2].transpose([0, 2, 1]),
                        Act.Copy, scale=0.5,
                    ).then_inc(s_act, 1)
                    act.activation(
                        PRB[:, 4:5, :], PR[:, :, 4:5].transpose([0, 2, 1]),
                        Act.Copy, scale=-1.0,
                    ).then_inc(s_act, 1)
                    for c in range(nchunk):
                        if skip_term1:
                            scalar.sem_inc(s_act, 1)
                            continue
                        scalar.wait_ge(dma_pc, 16 * (c + 1))
                        dfc = T[:, c * QC:(c + 1) * QC, :, :].rearrange(
                            "n q p v -> n q (p v)")
                        act.activation(
                            dfc, dfc, Act.Abs, accum_out=A1[:, c:c + 1],
                        ).then_inc(s_act, 1)
                    scalar.wait_ge(s_vec, V_Z)
                    act.activation(TPB[:], ZB[:], Act.Arctan).then_inc(s_act, 1)
                    # final per-partition sums via accum_out (off the DVE)
                    scalar.wait_ge(s_vec, V_D0)
                    if outlap and _it > 0:
                        # WAR: previous iteration's OUT DMA must have read OUT
                        scalar.wait_ge(dma_o, 16 * _it)
                    act.activation(
                        D2B[:, 0], D2B[:, 0], Act.Abs, accum_out=OUT[:, 1:2],
                    ).then_inc(s_act, 1)
                    act.activation(
                        A1[:], A1[:], Act.Copy, accum_out=OUT[:, 0:1],
                    ).then_inc(s_act, 1)
                    scalar.wait_ge(s_vec, V_D1)
                    act.activation(
                        D2B[:, 1, 0:3:2, :], D2B[:, 1, 0:3:2, :], Act.Abs,
                        accum_out=OUT[:, 2:3],
                    ).then_inc(s_act, 1)
                    act.activation(
                        D2B[:, 1, 1:2, :], D2B[:, 1, 1:2, :], Act.Abs,
                        accum_out=OUT[:, 3:4],
                    ).then_inc(s_act, 1)

                @block.gpsimd
                def _(gpsimd):
                    gp = nc.gpsimd

                    def cce(c):
                        if skip_term1:
                            gpsimd.sem_inc(dma_pc, 16)
                            return
                        qq = ((c + 1) * QC - 1) // Q4
                        gpsimd.wait_ge(s_act, MS_TAU(qq))
                        tchunk = T[:, c * QC:(c + 1) * QC, :, :].rearrange(
                            "n q p v -> n q (p v)")
                        gpsimd.dma_start(
                            out=tchunk,
                            in_=predR[:, c * QC:(c + 1) * QC, :],
                            accum_op=Alu.add,  # pred + (-tar), in place
                        ).then_inc(dma_pc, 16)

                    cce(0)
                    cce(1)
                    if QP == 0:
                        for c in range(2, nchunk):
                            cce(c)
                        gpsimd.sem_inc(s_gp, GP_CHAIN)
                    else:
                        gpsimd.wait_ge(s_vec, V_TRIG(0))
                        sl = slice(0, QP)
                        _closed_init(nc, gp, TRIG, TAU, V, U, sl)
                        last = None
                        for k in range(8, 0, -1):
                            pk = k - 1
                            last = _givens_step(nc, gp, TRIG, TAU, V, U,
                                                sl, pk, P, QP)
                            if k in (8, 7, 6):
                                c0 = 2 * (9 - k)
                                cce(c0)
                                cce(c0 + 1)
                        last.then_inc(s_gp, GP_CHAIN)

                @block.vector
                def _(vector):
                    vec = nc.vector
                    QD = Q - QP
                    for q in range(4):
                        lo, hi = qrange(q)
                        vector.wait_ge(s_act, MS_QSIN(q))
                        qs = QT[:, :, :, lo:hi]
                        hh = H[:, :, :, lo:hi]
                        c_ = TRIG[:, 0, :, :, lo:hi]
                        s_ = TRIG[:, 1, :, :, lo:hi]
                        ns_ = TRIG[:, 2, :, :, lo:hi]
                        # qs <- sin(x/4)^2 ; g' = 2cos(x/2) = -4*qs+2 (in place)
                        vec.tensor_tensor(out=qs, in0=qs, in1=qs, op=Alu.mult)
                        vec.tensor_scalar(qs, qs, -4.0, 2.0, Alu.mult, Alu.add)
                        vector.wait_ge(s_act, MS_SIN(q))
                        # C = 1 - 2*sin(x/2)^2  (square into TRIG0, then affine)
                        vec.tensor_tensor(out=c_, in0=hh, in1=hh, op=Alu.mult)
                        vec.tensor_scalar(c_, c_, -2.0, 1.0, Alu.mult, Alu.add)
                        # S = sin(x/2) * g'
                        vec.tensor_tensor(out=s_, in0=hh, in1=qs, op=Alu.mult)
                        vec.tensor_scalar(
                            ns_, s_, -1.0, None, Alu.mult
                        ).then_inc(s_vec, 1)
                    sl = slice(QP, Q)
                    _closed_init(nc, vec, TRIG, TAU, V, U, sl)
                    for k in range(8, 0, -1):
                        _givens_step(nc, vec, TRIG, TAU, V, U, sl, k - 1, P, QD)
                    vector.wait_ge(s_gp, GP_CHAIN)

                    # ---- tail (full Q), half-angle atan2 ----
                    c0v = lambda i: V[:, 0, i, :]
                    c1v = lambda i: V[:, 1, i, :]
                    # M22 = M00*M11 - M10*M01
                    vec.tensor_tensor(out=M2B[:, 0, :], in0=c0v(0), in1=c1v(1),
                                      op=Alu.mult)
                    vec.tensor_tensor(out=M2B[:, 1, :], in0=c0v(1), in1=c1v(0),
                                      op=Alu.mult)
                    vec.tensor_tensor(out=M2B[:, 2, :], in0=M2B[:, 0, :],
                                      in1=M2B[:, 1, :], op=Alu.subtract)
                    # w = max(1 - M20^2, tiny)  (bf16)
                    w = WB[:, 0, :]
                    t_ = WB[:, 1, :]
                    u_ = WB[:, 2, :]
                    vec.tensor_tensor(out=w, in0=c0v(2), in1=c0v(2), op=Alu.mult)
                    vec.tensor_scalar(w, w, -1.0, 1.0, Alu.mult, Alu.add)
                    vec.tensor_scalar(w, w, 1e-12, None, Alu.max)
                    # u = rsqrt(w): bf16 quake seed + 1 Newton
                    vec.tensor_scalar(
                        t_.bitcast(mybir.dt.int16), w.bitcast(mybir.dt.int16),
                        1, None, Alu.logical_shift_right)
                    vec.tensor_scalar(
                        u_.bitcast(mybir.dt.int16), t_.bitcast(mybir.dt.int16),
                        -1, 0x5F37, Alu.mult, Alu.add)
                    for _ in range(newton):
                        vec.tensor_tensor(out=t_, in0=u_, in1=u_, op=Alu.mult)
                        vec.tensor_tensor(out=t_, in0=t_, in1=w, op=Alu.mult)
                        vec.tensor_scalar(t_, t_, -0.5, 1.5, Alu.mult, Alu.add)
                        vec.tensor_tensor(out=u_, in0=u_, in1=t_, op=Alu.mult)
                    # cy = w * u  (bf16)
                    vec.tensor_tensor(out=CY[:, 0, :], in0=w, in1=u_, op=Alu.mult)
                    # DEN = [cy + M22, cy + M00]
                    vec.tensor_tensor(out=DEN[:, 0, :], in0=CY[:, 0, :],
                                      in1=M2B[:, 2, :], op=Alu.add)
                    vec.tensor_tensor(out=DEN[:, 1, :], in0=CY[:, 0, :],
                                      in1=c0v(0), op=Alu.add)
                    # clamp: DEN >= tiny (bf16 roundoff can drive cy+M near 0-)
                    vec.tensor_scalar(DEN[:], DEN[:], 6e-5, None, Alu.max)
                    # RC = 1/DEN: bf16 quake seed + 1 Newton
                    vec.tensor_scalar(
                        RC[:].bitcast(mybir.dt.int16),
                        DEN[:].bitcast(mybir.dt.int16),
                        -1, 0x7EF0, Alu.mult, Alu.add)
                    for _ in range(newton):
                        vec.tensor_tensor(out=RT[:], in0=DEN[:], in1=RC[:],
                                          op=Alu.mult)
                        vec.tensor_scalar(RT[:], RT[:], -1.0, 2.0,
                                          Alu.mult, Alu.add)
                        vec.tensor_tensor(out=RC[:], in0=RC[:], in1=RT[:],
                                          op=Alu.mult)
                    # Z = [M21*RC0, M20*u, M10*RC1]
                    vec.tensor_tensor(out=ZB[:, 0, :], in0=c1v(2),
                                      in1=RC[:, 0, :], op=Alu.mult)
                    vec.tensor_tensor(out=ZB[:, 1, :], in0=c0v(2), in1=u_,
                                      op=Alu.mult)
                    _z = vec.tensor_tensor(out=ZB[:, 2, :], in0=c0v(1),
                                           in1=RC[:, 1, :], op=Alu.mult)
                    _z.then_inc(s_vec, 1)  # V_Z -> ACT arctan
                    # tau diffs while arctan runs
                    vector.wait_ge(s_act, MS_PRB)
                    vec.tensor_tensor(
                        out=D2B[:, 0], in0=V[:, 2, :, :],
                        in1=PRB[:, 0:3, :], op=Alu.subtract,
                    ).then_inc(s_vec, 1)  # V_D0
                    vector.wait_ge(s_act, MS_ATAN)
                    vec.tensor_tensor(
                        out=D2B[:, 1], in0=TPB[:],
                        in1=PRB[:, 3:6, :], op=Alu.subtract,
                    ).then_inc(s_vec, 1)  # V_D1

            used = [nc.sync.engine, nc.gpsimd.engine, nc.scalar.engine,
                    nc.vector.engine]
            nc.multi_engine_barrier(used)
            import itertools
            # with preload, dma_t*/dma_pr values persist across iterations
            # (thresholds scale with the iteration index); still rearm the
            # DGE queues for every semaphore.
            keep = set()
            if preload:
                keep = {s.num for s in dma_t} | {dma_pr.num}
            if outlap:
                keep |= {dma_o.num}
            nums = sorted(s.num for s in all_sems)
            for _, grp in itertools.groupby(
                enumerate(nums), lambda t: t[1] - t[0]
            ):
                g = [n for _, n in grp]
                rng = range(g[0], g[-1] + 1)
                nc.gpsimd.dma_reset(rng)
                clr = [n for n in g if n not in keep]
                for _, grp2 in itertools.groupby(
                    enumerate(clr), lambda t: t[1] - t[0]
                ):
                    g2 = [n for _, n in grp2]
                    nc.gpsimd.sem_clear(range(g2[0], g2[-1] + 1))
            nc.multi_engine_barrier(used)

    return nc


def _closed_init(nc, eng, TRIG, TAU, V, U, sl):
    """V[:, :, :, sl] <- (col0(R9), col1(R9), tau9) in closed form."""
    C9 = lambda a: TRIG[:, 0, 8, a, sl]
    S9 = lambda a: TRIG[:, 1, 8, a, sl]
    u = lambda m, v, p: U[:, m, v, p, sl]
    tt = lambda out, i0, i1, op: eng.tensor_tensor(out=out, in0=i0, in1=i1, op=op)
    tt(u(0, 0, 0), S9(0), S9(1), Alu.mult)   # sxsy
    tt(u(0, 0, 1), C9(0), S9(1), Alu.mult)   # cxsy
    tt(V[:, 0, 0, sl], C9(1), C9(2), Alu.mult)   # cy*cz
    tt(u(0, 1, 0), C9(0), S9(2), Alu.mult)   # cx*sz
    tt(u(1, 0, 0), C9(1), S9(2), Alu.mult)   # cy*sz
    tt(u(0, 2, 0), S9(0), S9(2), Alu.mult)   # sx*sz
    tt(u(1, 0, 1), C9(0), C9(2), Alu.mult)   # cx*cz
    tt(u(1, 1, 1), S9(0), C9(2), Alu.mult)   # sx*cz
    tt(u(0, 1, 1), u(0, 0, 0), C9(2), Alu.mult)   # sxsy*cz
    tt(u(0, 2, 1), u(0, 0, 1), C9(2), Alu.mult)   # cxsy*cz
    tt(u(1, 1, 0), u(0, 0, 0), S9(2), Alu.mult)   # sxsy*sz
    tt(u(1, 2, 0), u(0, 0, 1), S9(2), Alu.mult)   # cxsy*sz
    eng.tensor_scalar_mul(V[:, 1, 0, sl], u(1, 0, 0), -1.0)
    tt(V[:, 0, 1, sl], u(0, 1, 0), u(0, 1, 1), Alu.add)
    tt(V[:, 0, 2, sl], u(0, 2, 0), u(0, 2, 1), Alu.subtract)
    tt(V[:, 1, 1, sl], u(1, 0, 1), u(1, 1, 0), Alu.subtract)
    tt(V[:, 1, 2, sl], u(1, 1, 1), u(1, 2, 0), Alu.add)
    eng.tensor_copy(out=V[:, 2, :, sl], in_=TAU[:, 8, :, sl])


def _givens_step(nc, eng, TRIG, TAU, V, U, sl, pk, P, Qn):
    """Apply pose pk's rotation (Rx@Ry@Rz) to V[:, :, :, sl], then add tau."""
    for (a, i0, i1, sigma) in AXIS_SPECS:
        d = i1 - i0
        pair = V[:, :, i0:i1 + 1:d, sl]
        cb = (TRIG[:, 0, pk, a, sl].unsqueeze(1)
              .unsqueeze(2).broadcast_to([P, 3, 2, Qn]))
        eng.tensor_tensor(out=U[:, 0, :, :, sl], in0=cb, in1=pair, op=Alu.mult)
        strig = (TRIG[:, 2:0:-1, pk, a, sl] if sigma < 0
                 else TRIG[:, 1:3, pk, a, sl])
        rpair = (V[:, :, 1::-1, sl] if (i0, i1) == (0, 1)
                 else V[:, :, 2::-2, sl] if (i0, i1) == (0, 2)
                 else V[:, :, 2:0:-1, sl])
        sb = strig.unsqueeze(1).broadcast_to([P, 3, 2, Qn])
        eng.tensor_tensor(out=U[:, 1, :, :, sl], in0=sb, in1=rpair, op=Alu.mult)
        eng.tensor_tensor(out=pair, in0=U[:, 0, :, :, sl],
                          in1=U[:, 1, :, :, sl], op=Alu.add)
    return eng.tensor_tensor(out=V[:, 2, :, sl], in0=V[:, 2, :, sl],
                             in1=TAU[:, pk, :, sl], op=Alu.add)


_NC_CACHE = {}

# proven-correct fast configuration (drains off in regions verified on HW)
FAST_V1_KW = dict(chain_drains=False, m2e_drains=False, trig_drains=False)
FAST_KW = dict(QP=0, newton=0, preload=1, outlap=1)
USE_V2 = True


def _get_nc(Q=256, nchunk=8):
    key = (Q, nchunk)
    if key not in _NC_CACHE:
        if USE_V2:
            _NC_CACHE[key] = build_nc_v2(Q, nchunk, **FAST_KW)
        else:
            _NC_CACHE[key] = build_nc(Q, nchunk, **FAST_V1_KW)
    return _NC_CACHE[key]


def kernel(pred, tar, pr_glpose, weight2):
    pred = np.asarray(pred, dtype=np.float32)
    tar = np.asarray(tar, dtype=np.float32)
    pr_glpose = np.asarray(pr_glpose, dtype=np.float32)
    weight2 = np.asarray(weight2, dtype=np.float32)

    Bs = pred.shape[0] // N_CORES
    nc = _get_nc(Q=Bs // P, nchunk=8)
    if USE_V2:
        tar = -tar  # v2 streams -tar so the CCE add computes pred - tar
    in_maps = [
        {
            "pred": np.ascontiguousarray(pred[i * Bs:(i + 1) * Bs]),
            "tar": np.ascontiguousarray(tar[i * Bs:(i + 1) * Bs]),
            "pr": np.ascontiguousarray(pr_glpose[i * Bs:(i + 1) * Bs]),
        }
        for i in range(N_CORES)
    ]
    sums = None
    for _attempt in range(3):
        res = run_bass_kernel_spmd(nc, in_maps, list(range(N_CORES)))
        partial = np.stack(
            [res.results[i]["out"] for i in range(N_CORES)])  # [8,128,2]
        sums = partial.astype(np.float64).sum(axis=(0, 1))
        if np.isfinite(sums).all():
            break
    assert sums is not None
    B = pred.shape[0]
    term1 = sums[0] / (B * 54)
    term2 = 0.1 * (sums[1] + 2.0 * sums[2] + sums[3]) / (B * 6)
    d = np.linalg.svd(weight2.astype(np.float64), compute_uv=False)
    term3 = 0.01 * np.mean(np.abs(d - 1.0))
    return np.float32(term1 + term2 + term3)

